# revision 19
# baseline (speedup 1.0000x reference)
"""PointNet feature propagation (3-NN interpolate + 2-layer pointwise MLP w/ BN)
as a Bass/Tile SPMD kernel for 8 Trainium2 NeuronCores.

Sharding: (B=2, N=16384) queries flattened to 32768 rows, 4096 per core
(core c -> batch c//4, query slice (c%4)*4096). Targets (S=4096) replicated
per batch. BatchNorm statistics are global over (B, N) -> two small in-kernel
AllReduces (one per MLP layer).
"""

import sys

if "/opt/trn_rl_repo" not in sys.path:
    sys.path.insert(0, "/opt/trn_rl_repo")

from contextlib import ExitStack

import ml_dtypes
import numpy as np

import concourse.bass as bass
from concourse import bacc
import concourse.mybir as mybir
import concourse.tile as tile
from concourse.bass import IndirectOffsetOnAxis
from concourse.bass_utils import run_bass_kernel_spmd

F32 = mybir.dt.float32
F32R = mybir.dt.float32r
BF16 = mybir.dt.bfloat16
U32 = mybir.dt.uint32
AF = mybir.ActivationFunctionType
ALU = mybir.AluOpType
AX = mybir.AxisListType

K = 3
EPS_KNN = 1e-8
EPS_BN = 1e-5

# Full-problem dims
B, N, S_FULL, DS, DT = 2, 16384, 4096, 128, 256
C_IN, C0, C1 = DS + DT, 256, 128
N_CORES = 8
Q_FULL = (B * N) // N_CORES  # 4096 queries per core


def build_program(
    n_cores=N_CORES,
    Q=Q_FULL,
    S=S_FULL,
    mm_dt=F32R,
    total_n=B * N,
    debug_taps=False,
):
    """Build the single-core Bass program (SPMD: same program on all cores)."""
    P = 128
    T = Q // P  # query tiles
    M0 = C0 // P  # 2 output-channel chunks for layer0
    K0 = C_IN // P  # 3 contraction chunks for layer0
    K1 = C0 // P  # 2 contraction chunks for layer1
    M1 = C1 // P  # 1
    HC = S // 512  # 512-wide matmul chunks of the score row
    NH = Q // 512  # 512-wide n chunks for MLP
    inv_n = 1.0 / float(total_n)


    nc = bacc.Bacc(
        "TRN2", target_bir_lowering=False, debug=False, num_devices=n_cores
    )

    # ---- I/O ----
    # Distance scores use a 3-way-split bf16 K=21 matmul:
    #   score = xs.xt - ||xt||^2/2  (exact to ~2^-27), d2 = ||xs||^2 - 2*score.
    # xs = a0 + a1 + a2 and xt = c0 + c1 + c2 (bf16 splits, host-prepared);
    # product terms kept: (0,0),(0,1),(1,0),(1,1),(0,2),(2,0).
    # xs_t rows: [1,1,1, a0, a0, a1, a1, a0, a2] (3 rows per block)
    xs_t_d = nc.dram_tensor("xs_t", [21, Q], BF16, kind="ExternalInput")
    xs_d = nc.dram_tensor("xs", [Q, 3], F32, kind="ExternalInput")
    # xt_t row 0: zeros; rows 1-3: xyz_t^T (fp32, for the ||xt||^2 computation)
    xt_t_d = nc.dram_tensor("xt_t", [4, S], F32, kind="ExternalInput")
    # xt_bf rows: [c0, c1, c0, c1, c2, c0]
    xt_bf_d = nc.dram_tensor("xt_bf", [18, S], BF16, kind="ExternalInput")
    fs_t_d = nc.dram_tensor("fs_t", [DS, Q], mm_dt, kind="ExternalInput")
    ft_d = nc.dram_tensor("ft", [S, DT], F32, kind="ExternalInput")
    w0t_d = nc.dram_tensor("w0t", [K0, P, C0], mm_dt, kind="ExternalInput")
    w1t_d = nc.dram_tensor("w1t", [K1, P, C1], mm_dt, kind="ExternalInput")
    b0_d = nc.dram_tensor("b0", [P, M0], F32, kind="ExternalInput")
    g0_d = nc.dram_tensor("g0", [P, M0], F32, kind="ExternalInput")
    be0_d = nc.dram_tensor("be0", [P, M0], F32, kind="ExternalInput")
    b1_d = nc.dram_tensor("b1", [P, M1], F32, kind="ExternalInput")
    g1_d = nc.dram_tensor("g1", [P, M1], F32, kind="ExternalInput")
    be1_d = nc.dram_tensor("be1", [P, M1], F32, kind="ExternalInput")
    ident_d = nc.dram_tensor("identity", [P, P], mm_dt, kind="ExternalInput")
    out_d = nc.dram_tensor("out", [C1, Q], F32, kind="ExternalOutput")

    if debug_taps:
        dbg_score = nc.dram_tensor("dbg_score", [P, S], F32, kind="ExternalOutput")
        dbg_mv = nc.dram_tensor("dbg_mv", [P, 8], F32, kind="ExternalOutput")
        dbg_mi = nc.dram_tensor("dbg_mi", [P, 8], U32, kind="ExternalOutput")
        dbg_wts = nc.dram_tensor("dbg_wts", [P, K], F32, kind="ExternalOutput")
        dbg_fi = nc.dram_tensor("dbg_fi", [P, Q], F32, kind="ExternalOutput")
        dbg_x0 = nc.dram_tensor("dbg_x0", [P, Q], F32, kind="ExternalOutput")
        dbg_st0 = nc.dram_tensor("dbg_st0", [P, 2 * M0], F32, kind="ExternalOutput")
        dbg_gst0 = nc.dram_tensor("dbg_gst0", [P, 2 * M0], F32, kind="ExternalOutput")

    # internal DRAM bounce buffers for the BN-stats AllReduces
    bn0_in = nc.dram_tensor("bn0_in", [P, 2 * M0], F32)
    bn0_out = nc.dram_tensor("bn0_out", [P, 2 * M0], F32)
    bn1_in = nc.dram_tensor("bn1_in", [P, 2 * M1], F32)
    bn1_out = nc.dram_tensor("bn1_out", [P, 2 * M1], F32)
    groups = [list(range(n_cores))]

    with ExitStack() as ctx:
        tc = ctx.enter_context(tile.TileContext(nc))
        const = ctx.enter_context(tc.tile_pool(name="const", bufs=1))
        big = ctx.enter_context(tc.tile_pool(name="big", bufs=3))
        small = ctx.enter_context(tc.tile_pool(name="small", bufs=4))
        gp = ctx.enter_context(tc.tile_pool(name="gp", bufs=4))
        gsp = ctx.enter_context(tc.tile_pool(name="gsp", bufs=4))
        ntbf = ctx.enter_context(tc.tile_pool(name="ntbf", bufs=2))
        ps_score = ctx.enter_context(tc.tile_pool(name="ps_score", bufs=2, space="PSUM"))
        ps_tp = ctx.enter_context(tc.tile_pool(name="ps_tp", bufs=2, space="PSUM"))
        ps_mlp = ctx.enter_context(tc.tile_pool(name="ps_mlp", bufs=2, space="PSUM"))

        # ================= setup =================
        # lhsT [21, Q] bf16 (host-prepared)
        lhsT = const.tile([21, Q], BF16, tag="lhsT")
        nc.sync.dma_start(lhsT[:, :], xs_t_d[:, :])

        # rhs [21, S] bf16: rows 0-2 = 3-way bf16 split of -||xt||^2/2,
        # rows 3-20 = host-packed coordinate split blocks
        rhs = const.tile([21, S], BF16, tag="rhs")
        nc.sync.dma_start(rhs[3:21, :], xt_bf_d[:, :])

        # mnt = -||xt||^2 / 2 in fp32 via a one-time K=4 fp32 matmul
        xt4 = const.tile([4, S], F32, tag="xt4")
        nc.sync.dma_start(xt4[:, :], xt_t_d[:, :])
        halves4 = const.tile([4, 1], F32, tag="halves4")
        nc.scalar.activation(halves4[:, :], xt4[:, 0:1], AF.Copy, scale=0.0, bias=0.5)
        nc.scalar.activation(xt4[:, :], xt4[:, :], AF.Square)  # in place; row 0 stays 0
        mnt = big.tile([1, S], F32, tag="big", name="mnt")
        for h in range(HC):
            ps_nt = ps_mlp.tile([1, 512], F32, tag="ps_m")
            nc.tensor.matmul(
                ps_nt[:, :],
                lhsT=halves4[:, :],
                rhs=xt4[:, bass.ts(h, 512)],
                start=True,
                stop=True,
            )
            nc.scalar.activation(
                mnt[:, bass.ts(h, 512)], ps_nt[:, :], AF.Copy, scale=-1.0
            )
        # 3-way bf16 split of mnt -> rhs rows 0-2 (DMA'd in: engine ops cannot
        # start at partitions 1/2)
        na_t = ntbf.tile([1, S], BF16, tag="ntbf", name="na_t")
        nc.scalar.activation(na_t[:, :], mnt[:, :], AF.Copy)
        na32 = big.tile([1, S], F32, tag="big", name="na32")
        nc.scalar.activation(na32[:, :], na_t[:, :], AF.Copy)
        r1 = big.tile([1, S], F32, tag="big", name="r1")
        nc.vector.tensor_tensor(r1[:, :], mnt[:, :], na32[:, :], op=ALU.subtract)
        nb_t = ntbf.tile([1, S], BF16, tag="ntbf", name="nb_t")
        nc.scalar.activation(nb_t[:, :], r1[:, :], AF.Copy)
        nb32 = big.tile([1, S], F32, tag="big", name="nb32")
        nc.scalar.activation(nb32[:, :], nb_t[:, :], AF.Copy)
        r2 = big.tile([1, S], F32, tag="big", name="r2")
        nc.vector.tensor_tensor(r2[:, :], r1[:, :], nb32[:, :], op=ALU.subtract)
        nc_t = ntbf.tile([1, S], BF16, tag="ntbf", name="nc_t")
        nc.scalar.activation(nc_t[:, :], r2[:, :], AF.Copy)
        nc.sync.dma_start(rhs[0:1, :], na_t[:, :])
        nc.sync.dma_start(rhs[1:2, :], nb_t[:, :])
        nc.sync.dma_start(rhs[2:3, :], nc_t[:, :])

        ident = const.tile([P, P], mm_dt, tag="ident")
        nc.sync.dma_start(ident[:, :], ident_d[:, :])

        # weights / bn params
        w0T = const.tile([P, K0 * C0], mm_dt, tag="w0T")
        for k in range(K0):
            nc.sync.dma_start(w0T[:, bass.ts(k, C0)], w0t_d[k])
        w1T = const.tile([P, K1 * C1], mm_dt, tag="w1T")
        for k in range(K1):
            nc.sync.dma_start(w1T[:, bass.ts(k, C1)], w1t_d[k])
        b0_sb = const.tile([P, M0], F32, tag="b0")
        g0_sb = const.tile([P, M0], F32, tag="g0")
        be0_sb = const.tile([P, M0], F32, tag="be0")
        b1_sb = const.tile([P, M1], F32, tag="b1")
        g1_sb = const.tile([P, M1], F32, tag="g1")
        be1_sb = const.tile([P, M1], F32, tag="be1")
        for sb, d in [
            (b0_sb, b0_d), (g0_sb, g0_d), (be0_sb, be0_d),
            (b1_sb, b1_d), (g1_sb, g1_d), (be1_sb, be1_d),
        ]:
            nc.sync.dma_start(sb[:, :], d[:, :])

        # feature rhs tiles for layer0: ft0 = feats_s^T, ft1/ft2 = feats_inter^T
        ftsb = [const.tile([P, Q], mm_dt, tag=f"ftsb{i}", name=f"ftsb{i}") for i in range(K0)]
        nc.sync.dma_start(ftsb[0][:, :], fs_t_d[:, :])

        # PE dep-nop: make the PE observe all DMA/GPSIMD-produced matmul inputs
        # once, so each later matmul needs at most one new semaphore wait
        # (walrus allows only one sync wait on a Matmult's LW struct).
        dep_nop = nc.tensor.nop(hint="dep").ins
        dep_nop.ins = [
            nc.tensor.lower_ap(a)
            for a in [
                lhsT[:, :], rhs[:, :], w0T[:, :], w1T[:, :],
                ident[:, :], ftsb[0][:, :],
            ]
        ]

        # ================= knn + interpolation loop =================
        for t in range(T):
            # distance scores: psum/sbuf [128 queries, S targets]
            score = big.tile([P, S], F32, tag="big")
            for hh in range(HC // 2):
                ps = ps_score.tile([P, 1024], F32, tag="ps_s")
                for j in range(2):
                    h = hh * 2 + j
                    nc.tensor.matmul(
                        ps[:, bass.ts(j, 512)],
                        lhsT=lhsT[:, bass.ts(t, P)],
                        rhs=rhs[:, bass.ts(h, 512)],
                        start=True,
                        stop=True,
                    )
                nc.scalar.activation(
                    score[:, bass.ts(hh, 1024)], ps[:, :], AF.Copy
                )

            # top-3 (max of score == min of d2)
            mv = small.tile([P, 8], F32, tag="mv")
            nc.vector.max(out=mv[:, :], in_=score[:, :])
            mi = small.tile([P, 8], U32, tag="mi")
            nc.vector.max_index(mi[:, :], mv[:, :], score[:, :])

            # ||xs||^2 for this tile
            xst = small.tile([P, 3], F32, tag="xst")
            nc.sync.dma_start(xst[:, :], xs_d[bass.ts(t, P), :])
            xsq = small.tile([P, 3], F32, tag="xsq")
            ns = small.tile([P, 1], F32, tag="ns")
            nc.scalar.activation(xsq[:, :], xst[:, :], AF.Square,
                                 scale=0.7071067811865476, accum_out=ns[:, :])

            # weights: w_k = (1/(d2_k+eps)) / sum_k
            d2e = small.tile([P, K], F32, tag="d2e")
            nc.vector.tensor_scalar(
                d2e[:, :], mv[:, 0:K], ns[:, :], None, op0=ALU.subtract
            )  # = score - ns = -d2
            nc.vector.tensor_scalar(
                d2e[:, :], d2e[:, :], -2.0, EPS_KNN, op0=ALU.mult, op1=ALU.add
            )  # = d2 + eps
            rec = small.tile([P, K], F32, tag="rec")
            nc.vector.reciprocal(rec[:, :], d2e[:, :])
            rsum = small.tile([P, 1], F32, tag="rsum")
            nc.vector.reduce_sum(rsum[:, :], rec[:, :], axis=AX.X)
            rsi = small.tile([P, 1], F32, tag="rsi")
            nc.vector.reciprocal(rsi[:, :], rsum[:, :])
            wts = small.tile([P, K], F32, tag="wts")
            nc.vector.tensor_scalar(
                wts[:, :], rec[:, :], rsi[:, :], None, op0=ALU.mult
            )

            if debug_taps and t == 0:
                nc.sync.dma_start(dbg_score[:, :], score[:, :])
                nc.sync.dma_start(dbg_mv[:, :], mv[:, :])
                nc.sync.dma_start(dbg_mi[:, :], mi[:, :])
                nc.sync.dma_start(dbg_wts[:, :], wts[:, :])

            # gather feats_t rows + scale by weight
            gs_tiles = []
            for k in range(K):
                g = gp.tile([P, DT], F32, tag="g")
                nc.gpsimd.indirect_dma_start(
                    out=g[:, :],
                    out_offset=None,
                    in_=ft_d[:, :],
                    in_offset=IndirectOffsetOnAxis(ap=mi[:, k : k + 1], axis=0),
                )
                gs = gsp.tile([P, DT], mm_dt, tag="gs")
                nc.scalar.activation(
                    gs[:, :], g[:, :], AF.Copy, scale=wts[:, k : k + 1]
                )
                gs_tiles.append(gs)

            # transpose-accumulate: feats_inter^T columns for this tile
            for d in range(DT // P):
                ps_t = ps_tp.tile([P, P], mm_dt, tag="ps_t")
                for k in range(K):
                    nc.tensor.matmul(
                        ps_t[:, :],
                        lhsT=gs_tiles[k][:, bass.ts(d, P)],
                        rhs=ident[:, :],
                        is_transpose=True,
                        start=(k == 0),
                        stop=(k == K - 1),
                    )
                nc.scalar.activation(
                    ftsb[1 + d][:, bass.ts(t, P)], ps_t[:, :], AF.Copy
                )

        # ================= MLP layer 0 =================
        x0 = [const.tile([P, Q], mm_dt, tag=f"x0_{m}", name=f"x0_{m}") for m in range(M0)]
        s0p = const.tile([P, M0 * NH], F32, tag="s0p")
        stats0 = const.tile([P, 2 * M0], F32, tag="stats0")
        for m in range(M0):
            for h in range(NH):
                ps = ps_mlp.tile([P, 512], F32, tag="ps_m")
                for k in range(K0):
                    nc.tensor.matmul(
                        ps[:, :],
                        lhsT=w0T[:, bass.ds(k * C0 + m * P, P)],
                        rhs=ftsb[k][:, bass.ts(h, 512)],
                        start=(k == 0),
                        stop=(k == K0 - 1),
                    )
                nc.scalar.activation(
                    x0[m][:, bass.ts(h, 512)],
                    ps[:, :],
                    AF.Identity,
                    bias=b0_sb[:, m : m + 1],
                    accum_out=s0p[:, m * NH + h : m * NH + h + 1],
                )
            # channel sums / sums of squares
            nc.vector.reduce_sum(
                stats0[:, m : m + 1], s0p[:, bass.ts(m, NH)], axis=AX.X
            )
            scr = big.tile([P, Q], F32, tag="big")
            nc.scalar.activation(
                scr[:, :], x0[m][:, :], AF.Square,
                accum_out=stats0[:, M0 + m : M0 + m + 1],
            )

        if debug_taps:
            nc.sync.dma_start(dbg_fi[:, :], ftsb[1][:, :].bitcast(F32))
            nc.sync.dma_start(dbg_x0[:, :], x0[0][:, :].bitcast(F32))
            nc.sync.dma_start(dbg_st0[:, :], stats0[:, :])

        # global stats
        nc.sync.dma_start(bn0_in[:, :], stats0[:, :])
        nc.gpsimd.collective_compute(
            "AllReduce", ALU.add, replica_groups=groups,
            ins=[bn0_in[:, :]], outs=[bn0_out[:, :]],
        )
        gstats0 = const.tile([P, 2 * M0], F32, tag="gstats0")
        nc.sync.dma_start(gstats0[:, :], bn0_out[:, :])
        if debug_taps:
            nc.sync.dma_start(dbg_gst0[:, :], gstats0[:, :])

        def bn_affine(gstats, g_sb, be_sb, m, M, scale_t, shift_t):
            mn = small.tile([P, 1], F32, tag="mn")
            nc.vector.tensor_scalar(mn[:, :], gstats[:, m : m + 1], inv_n, None, op0=ALU.mult)
            vv = small.tile([P, 1], F32, tag="vv")
            nc.vector.tensor_scalar(vv[:, :], gstats[:, M + m : M + m + 1], inv_n, None, op0=ALU.mult)
            mn2 = small.tile([P, 1], F32, tag="mn2")
            nc.vector.tensor_tensor(mn2[:, :], mn[:, :], mn[:, :], op=ALU.mult)
            var = small.tile([P, 1], F32, tag="var")
            nc.vector.tensor_tensor(var[:, :], vv[:, :], mn2[:, :], op=ALU.subtract)
            nc.vector.tensor_scalar(var[:, :], var[:, :], EPS_BN, None, op0=ALU.add)
            std = small.tile([P, 1], F32, tag="std")
            nc.scalar.activation(std[:, :], var[:, :], AF.Sqrt)
            sinv = small.tile([P, 1], F32, tag="sinv")
            nc.vector.reciprocal(sinv[:, :], std[:, :])
            nc.vector.tensor_tensor(
                scale_t[:, m : m + 1], g_sb[:, m : m + 1], sinv[:, :], op=ALU.mult
            )
            tmp = small.tile([P, 1], F32, tag="tmp")
            nc.vector.tensor_tensor(
                tmp[:, :], mn[:, :], scale_t[:, m : m + 1], op=ALU.mult
            )
            nc.vector.tensor_tensor(
                shift_t[:, m : m + 1], be_sb[:, m : m + 1], tmp[:, :], op=ALU.subtract
            )

        scale0 = const.tile([P, M0], F32, tag="scale0")
        shift0 = const.tile([P, M0], F32, tag="shift0")
        for m in range(M0):
            bn_affine(gstats0, g0_sb, be0_sb, m, M0, scale0, shift0)
        for m in range(M0):
            # h0 = relu(scale*x0 + shift), in place
            nc.scalar.activation(
                x0[m][:, :], x0[m][:, :], AF.Relu,
                bias=shift0[:, m : m + 1], scale=scale0[:, m : m + 1],
            )

        # ================= MLP layer 1 =================
        x1 = big.tile([P, Q], F32, tag="big")
        s1p = const.tile([P, NH], F32, tag="s1p")
        stats1 = const.tile([P, 2 * M1], F32, tag="stats1")
        for h in range(NH):
            ps = ps_mlp.tile([P, 512], F32, tag="ps_m")
            for k in range(K1):
                nc.tensor.matmul(
                    ps[:, :],
                    lhsT=w1T[:, bass.ts(k, C1)],
                    rhs=x0[k][:, bass.ts(h, 512)],
                    start=(k == 0),
                    stop=(k == K1 - 1),
                )
            nc.scalar.activation(
                x1[:, bass.ts(h, 512)],
                ps[:, :],
                AF.Identity,
                bias=b1_sb[:, 0:1],
                accum_out=s1p[:, h : h + 1],
            )
        nc.vector.reduce_sum(stats1[:, 0:1], s1p[:, :], axis=AX.X)
        scr1 = big.tile([P, Q], F32, tag="big")
        nc.scalar.activation(
            scr1[:, :], x1[:, :], AF.Square, accum_out=stats1[:, 1:2]
        )

        nc.sync.dma_start(bn1_in[:, :], stats1[:, :])
        nc.gpsimd.collective_compute(
            "AllReduce", ALU.add, replica_groups=groups,
            ins=[bn1_in[:, :]], outs=[bn1_out[:, :]],
        )
        gstats1 = const.tile([P, 2 * M1], F32, tag="gstats1")
        nc.sync.dma_start(gstats1[:, :], bn1_out[:, :])

        scale1 = const.tile([P, M1], F32, tag="scale1")
        shift1 = const.tile([P, M1], F32, tag="shift1")
        bn_affine(gstats1, g1_sb, be1_sb, 0, M1, scale1, shift1)
        nc.scalar.activation(
            x1[:, :], x1[:, :], AF.Relu,
            bias=shift1[:, 0:1], scale=scale1[:, 0:1],
        )
        nc.sync.dma_start(out_d[:, :], x1[:, :])

    nc.compile()
    return nc


def make_in_maps(inputs, n_cores=N_CORES, Q=Q_FULL, S=S_FULL):
    """Shard full inputs into per-core input maps (layout prep only)."""
    xyz_s = np.ascontiguousarray(np.asarray(inputs["xyz_s"], dtype=np.float32))
    xyz_t = np.ascontiguousarray(np.asarray(inputs["xyz_t"], dtype=np.float32))
    feats_s = np.ascontiguousarray(np.asarray(inputs["feats_s"], dtype=np.float32))
    feats_t = np.ascontiguousarray(np.asarray(inputs["feats_t"], dtype=np.float32))
    w0 = np.asarray(inputs["w0"], dtype=np.float32)
    w1 = np.asarray(inputs["w1"], dtype=np.float32)

    P = 128
    K0, M0 = C_IN // P, C0 // P
    K1, M1 = C0 // P, C1 // P
    w0t = np.ascontiguousarray(w0.T.reshape(K0, P, C0))
    w1t = np.ascontiguousarray(w1.T.reshape(K1, P, C1))
    b0 = np.ascontiguousarray(np.asarray(inputs["b0"], np.float32).reshape(M0, P).T)
    g0 = np.ascontiguousarray(np.asarray(inputs["gamma0"], np.float32).reshape(M0, P).T)
    be0 = np.ascontiguousarray(np.asarray(inputs["beta0"], np.float32).reshape(M0, P).T)
    b1 = np.ascontiguousarray(np.asarray(inputs["b1"], np.float32).reshape(M1, P).T)
    g1 = np.ascontiguousarray(np.asarray(inputs["gamma1"], np.float32).reshape(M1, P).T)
    be1 = np.ascontiguousarray(np.asarray(inputs["beta1"], np.float32).reshape(M1, P).T)

    shards_per_b = n_cores // xyz_s.shape[0]
    in_maps = []
    for c in range(n_cores):
        b, j = divmod(c, shards_per_b)
        n0 = j * Q
        xs = xyz_s[b, n0 : n0 + Q]
        bf = ml_dtypes.bfloat16

        def split3(m):
            p0 = m.astype(bf)
            r = m - p0.astype(np.float32)
            p1 = r.astype(bf)
            p2 = (r - p1.astype(np.float32)).astype(bf)
            return p0, p1, p2

        xsT = xs.T.astype(np.float32)
        a0, a1, a2 = split3(xsT)
        xs21 = np.empty((21, Q), bf)
        xs21[0:3] = np.ones((3, Q), bf)
        for i, blk in enumerate([a0, a0, a1, a1, a0, a2]):
            xs21[3 + 3 * i : 6 + 3 * i] = blk
        xtT = xyz_t[b].T.astype(np.float32)
        c0, c1, c2 = split3(xtT)
        xt_bf = np.concatenate([c0, c1, c0, c1, c2, c0], axis=0)
        xt4 = np.zeros((4, S), np.float32)
        xt4[1:4] = xtT
        in_maps.append(
            {
                "xs_t": xs21,
                "xs": np.ascontiguousarray(xs),
                "xt_t": xt4,
                "xt_bf": np.ascontiguousarray(xt_bf),
                "fs_t": np.ascontiguousarray(feats_s[b, n0 : n0 + Q].T),
                "ft": feats_t[b],
                "identity": np.eye(P, dtype=np.float32),
                "w0t": w0t,
                "w1t": w1t,
                "b0": b0,
                "g0": g0,
                "be0": be0,
                "b1": b1,
                "g1": g1,
                "be1": be1,
            }
        )
    return in_maps


_PROGRAM_CACHE = {}


def get_program():
    key = (N_CORES, Q_FULL, S_FULL)
    if key not in _PROGRAM_CACHE:
        _PROGRAM_CACHE[key] = build_program()
    return _PROGRAM_CACHE[key]


def kernel(**inputs):
    nc = get_program()
    in_maps = make_in_maps(inputs)
    res = run_bass_kernel_spmd(nc, in_maps, core_ids=list(range(N_CORES)))
    out = np.empty((B, N, C1), dtype=np.float32)
    shards_per_b = N_CORES // B
    for c in range(N_CORES):
        b, j = divmod(c, shards_per_b)
        n0 = j * Q_FULL
        out[b, n0 : n0 + Q_FULL, :] = res.results[c]["out"].T
    return out
